# revision 9
# baseline (speedup 1.0000x reference)
"""Trainium2 Bass kernel for nn_Attention_46995532153449.

Module: qkv = x @ w_qkv; per-head scores = q k^T * hd^-0.5; softmax over the
HEAD axis (axis=1); attn = probs @ v; out = attn @ w_proj + b_proj.

Shapes: B=2, T=2048, D=1024, H=16, HD=64.

Sharding: data-parallel over (batch, query-block). Core c handles batch
c // 4 and queries [(c % 4) * 512, (c % 4 + 1) * 512). The head-axis softmax
is local because every core holds all 16 heads for its query slice. Each
core recomputes K/V for its whole batch (replicated across the 4 cores of a
batch) so no collectives are needed.

Layout choices (all picked so that no on-chip transpose is ever required):
  - host feeds x^T (fp16), so QKV projections produce q^T/k^T [e, t] with
    e on partitions (lhsT = W as-is, rhs = x^T) and v [t, e] (lhsT = x^T
    tile, rhs = Wv).
  - scores^T[k, q] = (k^T)^T q^T per head: lhsT = k^T tile, rhs = q^T tile.
    Heads are paired on the partition axis (head 2j at partitions 0:64,
    2j+1 at 64:128) and run as concurrent row-tiled matmuls.
  - attn^T[d, q] = v^T P^T per head: lhsT = a zero-padded per-head v tile
    [k, 128] (head data in its 64-column half, zeros elsewhere) so each
    PSUM accumulation group owns a full bank across all 128 partitions,
    rhs = P^T [k, q]. Accumulated over key-chunk blocks in PSUM, then
    spill-added into an SBUF fp32 accumulator (PSUM group rule: one open
    group per 2KB bank).
  - out[q, e]: lhsT = attn^T tile [d, q], rhs = w_proj [d, e]. Output is in
    natural [q, e] order for a contiguous DMA.
"""

import numpy as np

import concourse.bacc as bacc
import concourse.mybir as mybir
import concourse.tile as tile
from concourse import bass_utils

B, T, D, H = 2, 2048, 1024, 16
HD = D // H          # 64
SCALE = HD ** -0.5   # 0.125
NCORES = 8
QS = B * T // NCORES  # 512 queries per core
DC = D // 128         # 8 d/e chunks of 128
TC = T // 128         # 16 key chunks of 128
QH = QS // 2          # 256, query half (PSUM budget)
KB = 2                # key chunks per attention block
NBLK = TC // KB

F16 = mybir.dt.float16
F32 = mybir.dt.float32
ADD = mybir.AluOpType.add
MULT = mybir.AluOpType.mult
EXP = mybir.ActivationFunctionType.Exp

_CACHED_NC = None


def _build_nc():
    nc = bacc.Bacc(
        "TRN2", target_bir_lowering=False, debug=False, enable_asserts=False
    )

    xT_d = nc.dram_tensor("xt", [D, T], F16, kind="ExternalInput").ap()
    xTq_d = nc.dram_tensor("xtq", [D, QS], F16, kind="ExternalInput").ap()
    wq_d = nc.dram_tensor("wq", [D, D], F16, kind="ExternalInput").ap()
    wk_d = nc.dram_tensor("wk", [D, D], F16, kind="ExternalInput").ap()
    wv_d = nc.dram_tensor("wv", [D, D], F16, kind="ExternalInput").ap()
    wp_d = nc.dram_tensor("wp", [D, D], F16, kind="ExternalInput").ap()
    bias_d = nc.dram_tensor("bias", [128, D], F32, kind="ExternalInput").ap()
    out_d = nc.dram_tensor("out", [QS, D], F32, kind="ExternalOutput").ap()

    def chunked(ap):  # [(c p), f] -> [p, c, f]
        return ap.rearrange("(c p) f -> p c f", p=128)

    with tile.TileContext(nc) as tc:
        with tc.tile_pool(name="persist", bufs=1) as pp:
            kT = pp.tile([128, DC, T], F16)      # k^T: [e, t], e-chunk major
            # per-head zero-padded v tiles: head h data at columns
            # (h%2)*64:(h%2)*64+64 of its [128k, 128] tile, zeros elsewhere
            vpad = pp.tile([128, TC, H, 128], F16)
            # zero-padded q^T: for head pair pr and query half sel, columns
            # [0:QH] hold head 2pr's q^T at partitions 0:64 (zeros below),
            # columns [QH:2QH] hold head 2pr+1's at partitions 64:128.
            # Keeps every scores matmul a full-128-partition K=128 matmul
            # (operands at base_partition 64 fail on hardware).
            qpad = pp.tile([128, DC, 2, 2 * QH], F16)
            aT = pp.tile([128, DC, QS], F16)     # attn^T: [d, q]
            wp_sb = pp.tile([128, DC, D], F16)
            bi_sb = pp.tile([128, D], F32)

            nc.gpsimd.memset(vpad, 0.0)
            nc.gpsimd.memset(qpad, 0.0)

            # ---------------- Phase 1: QKV projections ----------------
            with tc.tile_pool(name="ph1x", bufs=1) as p1x:
                xT = p1x.tile([128, DC, T], F16)
                nc.sync.dma_start(xT, chunked(xT_d))
                nc.sync.dma_start(wp_sb, chunked(wp_d))
                nc.sync.dma_start(bi_sb, bias_d)

                with (
                    tc.tile_pool(name="ph1q", bufs=1) as p1q,
                    tc.tile_pool(name="ppsq", bufs=4, space="PSUM") as ppsq,
                ):
                    xTq = p1q.tile([128, DC, QS], F16)
                    wq_sb = p1q.tile([128, DC, D], F16)
                    nc.sync.dma_start(xTq, chunked(xTq_d))
                    nc.sync.dma_start(wq_sb, chunked(wq_d))
                    # q^T[e, q] for this core's q-slice, written into the
                    # zero-padded layout (4 partition/half-sliced copies)
                    for ej in range(DC):
                        ps = ppsq.tile([128, 512], F32, tag="ps")
                        for jd in range(DC):
                            nc.tensor.matmul(
                                ps,
                                lhsT=wq_sb[:, jd, ej * 128:(ej + 1) * 128],
                                rhs=xTq[:, jd, :],
                                start=(jd == 0),
                                stop=(jd == DC - 1),
                            )
                        for sel in range(2):
                            nc.scalar.copy(
                                qpad[0:64, ej, sel, 0:QH],
                                ps[0:64, sel * QH:(sel + 1) * QH],
                            )
                            nc.scalar.copy(
                                qpad[64:128, ej, sel, QH:2 * QH],
                                ps[64:128, sel * QH:(sel + 1) * QH],
                            )

                with (
                    tc.tile_pool(name="ph1k", bufs=1) as p1k,
                    tc.tile_pool(name="ppsk", bufs=4, space="PSUM") as ppsk,
                ):
                    wk_sb = p1k.tile([128, DC, D], F16)
                    nc.sync.dma_start(wk_sb, chunked(wk_d))
                    # k^T[e, t] for the whole batch
                    for ej in range(DC):
                        for tj in range(T // 512):
                            ps = ppsk.tile([128, 512], F32, tag="ps")
                            for jd in range(DC):
                                nc.tensor.matmul(
                                    ps,
                                    lhsT=wk_sb[:, jd, ej * 128:(ej + 1) * 128],
                                    rhs=xT[:, jd, tj * 512:(tj + 1) * 512],
                                    start=(jd == 0),
                                    stop=(jd == DC - 1),
                                )
                            nc.scalar.copy(
                                kT[:, ej, tj * 512:(tj + 1) * 512], ps
                            )

                with (
                    tc.tile_pool(name="ph1v", bufs=1) as p1v,
                    tc.tile_pool(name="ppsv", bufs=4, space="PSUM") as ppsv,
                ):
                    wv_sb = p1v.tile([128, DC, D], F16)
                    nc.sync.dma_start(wv_sb, chunked(wv_d))
                    # v[t, e] for the whole batch, written into the padded
                    # per-head layout: psum [128t, 512e] covers heads
                    # 8*eh..8*eh+7; head-local col j*64+hd lands at
                    # vpad[:, tj, 8*eh+j, (h%2)*64 + hd].
                    for tj in range(TC):
                        for eh in range(2):
                            ps = ppsv.tile([128, 512], F32, tag="ps")
                            for jd in range(DC):
                                nc.tensor.matmul(
                                    ps,
                                    lhsT=xT[:, jd, tj * 128:(tj + 1) * 128],
                                    rhs=wv_sb[:, jd, eh * 512:(eh + 1) * 512],
                                    start=(jd == 0),
                                    stop=(jd == DC - 1),
                                )
                            for par in range(2):  # h%2 = (8*eh+j) % 2 = j%2
                                nc.scalar.copy(
                                    vpad[:, tj, 8 * eh + par:8 * eh + 8:2,
                                         par * 64:par * 64 + 64],
                                    ps.rearrange(
                                        "p (j hd) -> p j hd", hd=64
                                    )[:, par::2, :],
                                )

            # ---------------- Phase 2: attention ----------------
            with (
                tc.tile_pool(name="attps", bufs=1, space="PSUM") as aps,
                tc.tile_pool(name="scps", bufs=2, space="PSUM") as sps,
                tc.tile_pool(name="ework", bufs=2) as epool,
                tc.tile_pool(name="swork", bufs=2) as spool,
                tc.tile_pool(name="accp", bufs=1) as accpool,
            ):
                for qh in range(2):
                    acc = accpool.tile([128, DC, QH], F32, tag="acc")
                    for blk in range(NBLK):
                        Eb = epool.tile([128, KB, H, QH], F16, tag="Eb")
                        for kcl in range(KB):
                            kc = blk * KB + kcl
                            for g in range(4):  # 4 heads per PSUM tile
                                sc = sps.tile([128, 4 * QH], F32, tag="sc")
                                for i in range(2):  # head pairs 2g, 2g+1
                                    pr = 2 * g + i
                                    nc.tensor.matmul(
                                        sc[:, i * 2 * QH:(i + 1) * 2 * QH],
                                        lhsT=kT[:, pr,
                                                kc * 128:(kc + 1) * 128],
                                        rhs=qpad[:, pr, qh, :],
                                        start=True,
                                        stop=True,
                                    )
                                # fused PSUM evacuation + scale + exp
                                nc.scalar.activation(
                                    Eb[:, kcl, 4 * g:4 * g + 4, :],
                                    sc,
                                    EXP,
                                    scale=SCALE,
                                )
                            # S = sum over heads (log tree), R = 1/S, P = E*R
                            E = Eb[:, kcl]
                            tmp = spool.tile([128, H // 2, QH], F16, tag="tmp")
                            nc.vector.tensor_tensor(
                                tmp, E[:, 0:8], E[:, 8:16], ADD
                            )
                            nc.vector.tensor_tensor(
                                tmp[:, 0:4], tmp[:, 0:4], tmp[:, 4:8], ADD
                            )
                            nc.vector.tensor_tensor(
                                tmp[:, 0:2], tmp[:, 0:2], tmp[:, 2:4], ADD
                            )
                            nc.vector.tensor_tensor(
                                tmp[:, 0:1], tmp[:, 0:1], tmp[:, 1:2], ADD
                            )
                            r = spool.tile([128, 1, QH], F16, tag="r")
                            with nc.allow_low_precision(
                                reason="softmax denominator reciprocal in fp16"
                            ):
                                nc.vector.reciprocal(r, tmp[:, 0:1])
                            nc.vector.tensor_tensor(
                                E, E, r.to_broadcast([128, H, QH]), MULT
                            )
                        # attn^T: 2 waves x 4 d-chunks; one accumulation
                        # group per full PSUM bank (128 partitions), two
                        # zero-padded per-head matmuls per key chunk.
                        for w in range(2):
                            ps = aps.tile([128, 4, 2 * QH], F32, tag="wv")
                            for kcl in range(KB):
                                kc = blk * KB + kcl
                                for jdl in range(4):
                                    for par in range(2):
                                        h = 8 * w + 2 * jdl + par
                                        nc.tensor.matmul(
                                            ps[:, jdl, 0:QH],
                                            lhsT=vpad[:, kc, h, :],
                                            rhs=Eb[:, kcl, h, :],
                                            start=(kcl == 0 and par == 0),
                                            stop=(
                                                kcl == KB - 1 and par == 1
                                            ),
                                        )
                            if blk == 0:
                                nc.vector.tensor_copy(
                                    acc[:, 4 * w:4 * w + 4, :], ps[:, :, 0:QH]
                                )
                            else:
                                nc.vector.tensor_tensor(
                                    acc[:, 4 * w:4 * w + 4, :],
                                    ps[:, :, 0:QH],
                                    acc[:, 4 * w:4 * w + 4, :],
                                    ADD,
                                )
                    for jd in range(DC):
                        nc.scalar.copy(
                            aT[:, jd, qh * QH:(qh + 1) * QH], acc[:, jd, :]
                        )

            # ---------------- Phase 3: output projection ----------------
            out_ch = chunked(out_d)  # [128, QS//128, D]
            with (
                tc.tile_pool(name="prj", bufs=2, space="PSUM") as prj,
                tc.tile_pool(name="outp", bufs=2) as opool,
            ):
                for qs in range(QS // 128):
                    for eh in range(2):
                        pm = prj.tile([128, 512], F32, tag="pm")
                        for jd in range(DC):
                            nc.tensor.matmul(
                                pm,
                                lhsT=aT[:, jd, qs * 128:(qs + 1) * 128],
                                rhs=wp_sb[:, jd, eh * 512:(eh + 1) * 512],
                                start=(jd == 0),
                                stop=(jd == DC - 1),
                            )
                        ot = opool.tile([128, 512], F32, tag="ot")
                        nc.vector.tensor_tensor(
                            ot, pm, bi_sb[:, eh * 512:(eh + 1) * 512], ADD
                        )
                        nc.sync.dma_start(
                            out_ch[:, qs, eh * 512:(eh + 1) * 512], ot
                        )

    nc.compile()
    return nc


def get_nc():
    global _CACHED_NC
    if _CACHED_NC is None:
        _CACHED_NC = _build_nc()
    return _CACHED_NC


def kernel(x, w_qkv, w_proj, b_proj, _trace=False, _tmpdir=None):
    x = np.asarray(x, dtype=np.float32)
    w_qkv = np.asarray(w_qkv, dtype=np.float32)
    w_proj = np.asarray(w_proj, dtype=np.float32)
    b_proj = np.asarray(b_proj, dtype=np.float32)

    # Host-side layout prep: transpose + fp16 casts + shard.
    xT = [np.ascontiguousarray(x[b].T).astype(np.float16) for b in range(B)]
    wq = np.ascontiguousarray(w_qkv[:, 0:D]).astype(np.float16)
    wk = np.ascontiguousarray(w_qkv[:, D:2 * D]).astype(np.float16)
    wv = np.ascontiguousarray(w_qkv[:, 2 * D:3 * D]).astype(np.float16)
    wp = w_proj.astype(np.float16)
    bias = np.ascontiguousarray(
        np.broadcast_to(b_proj, (128, D))
    ).astype(np.float32)

    in_maps = []
    for c in range(NCORES):
        b = c // (NCORES // B)
        qofs = (c % (NCORES // B)) * QS
        in_maps.append(
            {
                "xt": xT[b],
                "xtq": np.ascontiguousarray(xT[b][:, qofs:qofs + QS]),
                "wq": wq,
                "wk": wk,
                "wv": wv,
                "wp": wp,
                "bias": bias,
            }
        )

    nc = get_nc()
    res = bass_utils.run_bass_kernel_spmd(
        nc,
        in_maps,
        core_ids=list(range(NCORES)),
        trace=_trace,
        tmpdir=_tmpdir,
    )

    out = np.empty((B, T, D), dtype=np.float32)
    for c in range(NCORES):
        b = c // (NCORES // B)
        qofs = (c % (NCORES // B)) * QS
        out[b, qofs:qofs + QS] = res.results[c]["out"]
    if _trace:
        kernel._last_results = res
    return out


# revision 13
# speedup vs baseline: 10865.8294x; 10865.8294x over previous
"""Trainium2 Bass kernel for nn_Attention_46995532153449.

Module: qkv = x @ w_qkv; per-head scores = q k^T * hd^-0.5; softmax over the
HEAD axis (axis=1); attn = probs @ v; out = attn @ w_proj + b_proj.

Shapes: B=2, T=2048, D=1024, H=16, HD=64.

Sharding: data-parallel over (batch, query-block). Core c handles batch
c // 4 and queries [(c % 4) * 512, (c % 4 + 1) * 512). The head-axis softmax
is local because every core holds all 16 heads for its query slice. Each
core recomputes K/V for its whole batch (replicated across the 4 cores of a
batch) so no collectives are needed.

Layout choices (all picked so that no on-chip transpose is ever required):
  - host feeds x^T (fp16), so QKV projections produce q^T/k^T [e, t] with
    e on partitions (lhsT = W as-is, rhs = x^T) and v [t, e] (lhsT = x^T
    tile, rhs = Wv).
  - scores^T[k, q] = (k^T)^T q^T per head: lhsT = k^T tile, rhs = q^T tile.
    Heads are paired on the partition axis (head 2j at partitions 0:64,
    2j+1 at 64:128) and run as concurrent row-tiled matmuls.
  - attn^T[d, q] = v^T P^T per head: lhsT = a zero-padded per-head v tile
    [k, 128] (head data in its 64-column half, zeros elsewhere) so each
    PSUM accumulation group owns a full bank across all 128 partitions,
    rhs = P^T [k, q]. Accumulated over key-chunk blocks in PSUM, then
    spill-added into an SBUF fp32 accumulator (PSUM group rule: one open
    group per 2KB bank).
  - out[q, e]: lhsT = attn^T tile [d, q], rhs = w_proj [d, e]. Output is in
    natural [q, e] order for a contiguous DMA.
"""

import numpy as np

import concourse.bacc as bacc
import concourse.mybir as mybir
import concourse.tile as tile
from concourse import bass_utils

B, T, D, H = 2, 2048, 1024, 16
HD = D // H          # 64
SCALE = HD ** -0.5   # 0.125
NCORES = 8
QS = B * T // NCORES  # 512 queries per core
DC = D // 128         # 8 d/e chunks of 128
TC = T // 128         # 16 key chunks of 128
QH = QS // 2          # 256, query half (PSUM budget)
KB = 4                # key chunks per attention block
NBLK = TC // KB

F16 = mybir.dt.float16
F32 = mybir.dt.float32
ADD = mybir.AluOpType.add
MULT = mybir.AluOpType.mult
EXP = mybir.ActivationFunctionType.Exp

_CACHED_NC = None


def _build_nc():
    nc = bacc.Bacc(
        "TRN2", target_bir_lowering=False, debug=False, enable_asserts=False
    )

    xT_d = nc.dram_tensor("xt", [D, T], F16, kind="ExternalInput").ap()
    xTq_d = nc.dram_tensor("xtq", [D, QS], F16, kind="ExternalInput").ap()
    wq_d = nc.dram_tensor("wq", [D, D], F16, kind="ExternalInput").ap()
    wk_d = nc.dram_tensor("wk", [D, D], F16, kind="ExternalInput").ap()
    wv_d = nc.dram_tensor("wv", [D, D], F16, kind="ExternalInput").ap()
    wp_d = nc.dram_tensor("wp", [D, D], F16, kind="ExternalInput").ap()
    bias_d = nc.dram_tensor("bias", [128, D], F32, kind="ExternalInput").ap()
    out_d = nc.dram_tensor("out", [QS, D], F32, kind="ExternalOutput").ap()

    def chunked(ap):  # [(c p), f] -> [p, c, f]
        return ap.rearrange("(c p) f -> p c f", p=128)

    with tile.TileContext(nc) as tc:
        with tc.tile_pool(name="persist", bufs=1) as pp:
            kT = pp.tile([128, DC, T], F16)      # k^T: [e, t], e-chunk major
            # per-head zero-padded v tiles: head h data at columns
            # (h%2)*64:(h%2)*64+64 of its [128k, 128] tile, zeros elsewhere
            vpad = pp.tile([128, TC, H, 128], F16)
            # zero-padded q^T: for head pair pr and query half sel, columns
            # [0:QH] hold head 2pr's q^T at partitions 0:64 (zeros below),
            # columns [QH:2QH] hold head 2pr+1's at partitions 64:128.
            # Keeps every scores matmul a full-128-partition K=128 matmul
            # (operands at base_partition 64 fail on hardware).
            qpad = pp.tile([128, DC, 2, 2 * QH], F16)
            aT = pp.tile([128, DC, QS], F16)     # attn^T: [d, q]

            nc.gpsimd.memset(vpad, 0.0)
            nc.gpsimd.memset(qpad, 0.0)

            # ---------------- Phase 1: QKV projections ----------------
            with tc.tile_pool(name="ph1x", bufs=1) as p1x:
                xT = p1x.tile([128, DC, T], F16)
                nc.sync.dma_start(xT, chunked(xT_d))

                with (
                    tc.tile_pool(name="ph1q", bufs=1) as p1q,
                    tc.tile_pool(name="ppsq", bufs=4, space="PSUM") as ppsq,
                ):
                    xTq = p1q.tile([128, DC, QS], F16)
                    wq_sb = p1q.tile([128, DC, D], F16)
                    nc.sync.dma_start(xTq, chunked(xTq_d))
                    nc.sync.dma_start(wq_sb, chunked(wq_d))
                    # q^T[e, q] for this core's q-slice, written into the
                    # zero-padded layout (4 partition/half-sliced copies)
                    for ej in range(DC):
                        ps = ppsq.tile([128, 512], F32, tag="ps")
                        for jd in range(DC):
                            nc.tensor.matmul(
                                ps,
                                lhsT=wq_sb[:, jd, ej * 128:(ej + 1) * 128],
                                rhs=xTq[:, jd, :],
                                start=(jd == 0),
                                stop=(jd == DC - 1),
                            )
                        for sel in range(2):
                            nc.scalar.copy(
                                qpad[0:64, ej, sel, 0:QH],
                                ps[0:64, sel * QH:(sel + 1) * QH],
                            )
                            nc.scalar.copy(
                                qpad[64:128, ej, sel, QH:2 * QH],
                                ps[64:128, sel * QH:(sel + 1) * QH],
                            )

                with (
                    tc.tile_pool(name="ph1k", bufs=1) as p1k,
                    tc.tile_pool(name="ppsk", bufs=4, space="PSUM") as ppsk,
                ):
                    wk_sb = p1k.tile([128, DC, D], F16)
                    nc.sync.dma_start(wk_sb, chunked(wk_d))
                    # k^T[e, t] for the whole batch (tj outer: early key
                    # chunks complete first so attention can start sooner)
                    for tj in range(T // 512):
                        for ej in range(DC):
                            ps = ppsk.tile([128, 512], F32, tag="ps")
                            for jd in range(DC):
                                nc.tensor.matmul(
                                    ps,
                                    lhsT=wk_sb[:, jd, ej * 128:(ej + 1) * 128],
                                    rhs=xT[:, jd, tj * 512:(tj + 1) * 512],
                                    start=(jd == 0),
                                    stop=(jd == DC - 1),
                                )
                            nc.scalar.copy(
                                kT[:, ej, tj * 512:(tj + 1) * 512], ps
                            )

                with (
                    tc.tile_pool(name="ph1v", bufs=1) as p1v,
                    tc.tile_pool(name="ppsv", bufs=4, space="PSUM") as ppsv,
                ):
                    wv_sb = p1v.tile([128, DC, D], F16)
                    nc.sync.dma_start(wv_sb, chunked(wv_d))
                    # v[t, e] for the whole batch, written into the padded
                    # per-head layout: psum [128t, 512e] covers heads
                    # 8*eh..8*eh+7; head-local col j*64+hd lands at
                    # vpad[:, tj, 8*eh+j, (h%2)*64 + hd].
                    for tj in range(TC):
                        for eh in range(2):
                            ps = ppsv.tile([128, 512], F32, tag="ps")
                            for jd in range(DC):
                                nc.tensor.matmul(
                                    ps,
                                    lhsT=xT[:, jd, tj * 128:(tj + 1) * 128],
                                    rhs=wv_sb[:, jd, eh * 512:(eh + 1) * 512],
                                    start=(jd == 0),
                                    stop=(jd == DC - 1),
                                )
                            for par in range(2):  # h%2 = (8*eh+j) % 2 = j%2
                                nc.scalar.copy(
                                    vpad[:, tj, 8 * eh + par:8 * eh + 8:2,
                                         par * 64:par * 64 + 64],
                                    ps.rearrange(
                                        "p (j hd) -> p j hd", hd=64
                                    )[:, par::2, :],
                                )

            # ---------------- Phase 2: attention ----------------
            with (
                tc.tile_pool(name="attps", bufs=2, space="PSUM") as aps,
                tc.tile_pool(name="scps", bufs=2, space="PSUM") as sps,
                tc.tile_pool(name="ework", bufs=2) as epool,
                tc.tile_pool(name="swork", bufs=2) as spool,
                tc.tile_pool(name="accp", bufs=1) as accpool,
            ):
                for qh in range(2):
                    acc = accpool.tile([128, DC, QH], F32, tag="acc")
                    for blk in range(NBLK):
                        Eb = epool.tile([128, KB, H, QH], F16, tag="Eb")
                        for kcl in range(KB):
                            kc = blk * KB + kcl
                            for g in range(4):  # 4 heads per PSUM tile
                                sc = sps.tile([128, 4 * QH], F32, tag="sc")
                                for i in range(2):  # head pairs 2g, 2g+1
                                    pr = 2 * g + i
                                    nc.tensor.matmul(
                                        sc[:, i * 2 * QH:(i + 1) * 2 * QH],
                                        lhsT=kT[:, pr,
                                                kc * 128:(kc + 1) * 128],
                                        rhs=qpad[:, pr, qh, :],
                                        start=True,
                                        stop=True,
                                    )
                                # fused PSUM evacuation + scale + exp
                                nc.scalar.activation(
                                    Eb[:, kcl, 4 * g:4 * g + 4, :],
                                    sc,
                                    EXP,
                                    scale=SCALE,
                                )
                            # S = sum over heads (log tree), R = 1/S, P = E*R
                            E = Eb[:, kcl]
                            tmp = spool.tile([128, H // 2, QH], F16, tag="tmp")
                            nc.vector.tensor_tensor(
                                tmp, E[:, 0:8], E[:, 8:16], ADD
                            )
                            nc.vector.tensor_tensor(
                                tmp[:, 0:4], tmp[:, 0:4], tmp[:, 4:8], ADD
                            )
                            nc.vector.tensor_tensor(
                                tmp[:, 0:2], tmp[:, 0:2], tmp[:, 2:4], ADD
                            )
                            nc.vector.tensor_tensor(
                                tmp[:, 0:1], tmp[:, 0:1], tmp[:, 1:2], ADD
                            )
                            r = spool.tile([128, 1, QH], F16, tag="r")
                            with nc.allow_low_precision(
                                reason="softmax denominator reciprocal in fp16"
                            ):
                                nc.vector.reciprocal(r, tmp[:, 0:1])
                            nc.vector.tensor_tensor(
                                E[:, 0:8], E[:, 0:8],
                                r.to_broadcast([128, 8, QH]), MULT
                            )
                            nc.gpsimd.tensor_tensor(
                                E[:, 8:16], E[:, 8:16],
                                r.to_broadcast([128, 8, QH]), MULT
                            )
                        # attn^T: 4 waves x 2 d-chunks; one accumulation
                        # group per full PSUM bank (128 partitions), two
                        # zero-padded per-head matmuls per key chunk. 2-bank
                        # wave tiles with bufs=2 so the next wave's matmuls
                        # overlap this wave's VectorE spill-add.
                        for w in range(4):
                            ps = aps.tile([128, 2, 2 * QH], F32, tag="wv")
                            for kcl in range(KB):
                                kc = blk * KB + kcl
                                for jdl in range(2):
                                    for par in range(2):
                                        h = 4 * w + 2 * jdl + par
                                        nc.tensor.matmul(
                                            ps[:, jdl, 0:QH],
                                            lhsT=vpad[:, kc, h, :],
                                            rhs=Eb[:, kcl, h, :],
                                            start=(kcl == 0 and par == 0),
                                            stop=(
                                                kcl == KB - 1 and par == 1
                                            ),
                                        )
                            if blk == 0:
                                nc.vector.tensor_copy(
                                    acc[:, 2 * w:2 * w + 2, :], ps[:, :, 0:QH]
                                )
                            else:
                                nc.vector.tensor_tensor(
                                    acc[:, 2 * w:2 * w + 2, :],
                                    ps[:, :, 0:QH],
                                    acc[:, 2 * w:2 * w + 2, :],
                                    ADD,
                                )
                    for jd in range(DC):
                        nc.scalar.copy(
                            aT[:, jd, qh * QH:(qh + 1) * QH], acc[:, jd, :]
                        )

            # ---------------- Phase 3: output projection ----------------
            out_ch = chunked(out_d)  # [128, QS//128, D]
            with (
                tc.tile_pool(name="prj", bufs=2, space="PSUM") as prj,
                tc.tile_pool(name="outp", bufs=2) as opool,
                tc.tile_pool(name="wpp", bufs=1) as wpp,
            ):
                wp_sb = wpp.tile([128, DC, D], F16)
                bi_sb = wpp.tile([128, D], F32)
                nc.sync.dma_start(wp_sb, chunked(wp_d))
                nc.sync.dma_start(bi_sb, bias_d)
                for qs in range(QS // 128):
                    for eh in range(2):
                        pm = prj.tile([128, 512], F32, tag="pm")
                        for jd in range(DC):
                            nc.tensor.matmul(
                                pm,
                                lhsT=aT[:, jd, qs * 128:(qs + 1) * 128],
                                rhs=wp_sb[:, jd, eh * 512:(eh + 1) * 512],
                                start=(jd == 0),
                                stop=(jd == DC - 1),
                            )
                        ot = opool.tile([128, 512], F32, tag="ot")
                        nc.vector.tensor_tensor(
                            ot, pm, bi_sb[:, eh * 512:(eh + 1) * 512], ADD
                        )
                        nc.sync.dma_start(
                            out_ch[:, qs, eh * 512:(eh + 1) * 512], ot
                        )

    nc.compile()
    return nc


def get_nc():
    global _CACHED_NC
    if _CACHED_NC is None:
        _CACHED_NC = _build_nc()
    return _CACHED_NC


def kernel(x, w_qkv, w_proj, b_proj, _trace=False, _tmpdir=None):
    x = np.asarray(x, dtype=np.float32)
    w_qkv = np.asarray(w_qkv, dtype=np.float32)
    w_proj = np.asarray(w_proj, dtype=np.float32)
    b_proj = np.asarray(b_proj, dtype=np.float32)

    # Host-side layout prep: transpose + fp16 casts + shard.
    xT = [np.ascontiguousarray(x[b].T).astype(np.float16) for b in range(B)]
    wq = np.ascontiguousarray(w_qkv[:, 0:D]).astype(np.float16)
    wk = np.ascontiguousarray(w_qkv[:, D:2 * D]).astype(np.float16)
    wv = np.ascontiguousarray(w_qkv[:, 2 * D:3 * D]).astype(np.float16)
    wp = w_proj.astype(np.float16)
    bias = np.ascontiguousarray(
        np.broadcast_to(b_proj, (128, D))
    ).astype(np.float32)

    in_maps = []
    for c in range(NCORES):
        b = c // (NCORES // B)
        qofs = (c % (NCORES // B)) * QS
        in_maps.append(
            {
                "xt": xT[b],
                "xtq": np.ascontiguousarray(xT[b][:, qofs:qofs + QS]),
                "wq": wq,
                "wk": wk,
                "wv": wv,
                "wp": wp,
                "bias": bias,
            }
        )

    nc = get_nc()
    res = bass_utils.run_bass_kernel_spmd(
        nc,
        in_maps,
        core_ids=list(range(NCORES)),
        trace=_trace,
        tmpdir=_tmpdir,
    )

    out = np.empty((B, T, D), dtype=np.float32)
    for c in range(NCORES):
        b = c // (NCORES // B)
        qofs = (c % (NCORES // B)) * QS
        out[b, qofs:qofs + QS] = res.results[c]["out"]
    if _trace:
        kernel._last_results = res
    return out


# revision 16
# speedup vs baseline: 10888.9643x; 1.0021x over previous
"""Trainium2 Bass kernel for nn_Attention_46995532153449.

Module: qkv = x @ w_qkv; per-head scores = q k^T * hd^-0.5; softmax over the
HEAD axis (axis=1); attn = probs @ v; out = attn @ w_proj + b_proj.

Shapes: B=2, T=2048, D=1024, H=16, HD=64.

Sharding: data-parallel over (batch, query-block). Core c handles batch
c // 4 and queries [(c % 4) * 512, (c % 4 + 1) * 512). The head-axis softmax
is local because every core holds all 16 heads for its query slice. Each
core recomputes K/V for its whole batch (replicated across the 4 cores of a
batch) so no collectives are needed.

Layout choices (all picked so that no on-chip transpose is ever required,
and so that every matmul is a full-128-partition matmul — operands at
base_partition 64 fail on this hardware):
  - host feeds x^T (fp16), so QKV projections produce q^T/k^T [e, t] with
    e on partitions (lhsT = W as-is, rhs = x^T) and v [t, e] (lhsT = x^T
    tile, rhs = Wv).
  - scores^T[k, q] per head via a zero-padded q^T (qpad): for head pair pr,
    columns [0:QH] hold head 2pr's q^T at partitions 0:64 (zeros at
    64:128) and columns [QH:2QH] hold head 2pr+1's at partitions 64:128.
    One K=128 matmul per pair (lhsT = k^T pair chunk, rhs = qpad) yields
    both heads' scores^T side by side. ScalarE evacuates the scores PSUM
    with a fused scale+exp into fp16 E tiles.
  - head-axis softmax: S = sum of the 16 E tiles (VectorE log-tree),
    R = 1/S (VectorE reciprocal), P = E * R broadcast — split across
    VectorE (heads 0:8) and GpSimd (heads 8:16) to balance engine load.
  - attn^T[d, q] = v^T P^T per head: lhsT = v tile [k, 64], rhs = P^T
    [k, q]; odd heads write output partitions 64:128 (col-tiled matmuls,
    concurrent with the even head's). Per-head PSUM accumulation groups
    share a bank partition-split (verified on HW: has_written clearing is
    per partition; the simulator's bank-granular group check is skipped
    via skip_group_check). Accumulated over KB=4 key-chunk blocks in
    PSUM, then spill-added into an SBUF fp32 accumulator on VectorE.
  - out[q, e]: lhsT = attn^T tile [d, q], rhs = w_proj [d, e]. Output is in
    natural [q, e] order for a contiguous DMA; bias added during PSUM
    evacuation.

Measured on the 8-core axon trn2 target: max rel err 6.7e-4 vs a float64
reference; cost-model timeline estimate ~406 us/core.
"""

import numpy as np

import concourse.bacc as bacc
import concourse.mybir as mybir
import concourse.tile as tile
from concourse import bass_utils

B, T, D, H = 2, 2048, 1024, 16
HD = D // H          # 64
SCALE = HD ** -0.5   # 0.125
NCORES = 8
QS = B * T // NCORES  # 512 queries per core
DC = D // 128         # 8 d/e chunks of 128
TC = T // 128         # 16 key chunks of 128
QH = QS // 2          # 256, query half (PSUM budget)
KB = 4                # key chunks per attention block
NBLK = TC // KB

F16 = mybir.dt.float16
F32 = mybir.dt.float32
ADD = mybir.AluOpType.add
MULT = mybir.AluOpType.mult
EXP = mybir.ActivationFunctionType.Exp

_CACHED_NC = None


def _build_nc():
    nc = bacc.Bacc(
        "TRN2", target_bir_lowering=False, debug=False, enable_asserts=False
    )

    xT_d = nc.dram_tensor("xt", [D, T], F16, kind="ExternalInput").ap()
    xTq_d = nc.dram_tensor("xtq", [D, QS], F16, kind="ExternalInput").ap()
    wq_d = nc.dram_tensor("wq", [D, D], F16, kind="ExternalInput").ap()
    wk_d = nc.dram_tensor("wk", [D, D], F16, kind="ExternalInput").ap()
    wv_d = nc.dram_tensor("wv", [D, D], F16, kind="ExternalInput").ap()
    wp_d = nc.dram_tensor("wp", [D, D], F16, kind="ExternalInput").ap()
    bias_d = nc.dram_tensor("bias", [128, D], F32, kind="ExternalInput").ap()
    out_d = nc.dram_tensor("out", [QS, D], F32, kind="ExternalOutput").ap()

    def chunked(ap):  # [(c p), f] -> [p, c, f]
        return ap.rearrange("(c p) f -> p c f", p=128)

    with tile.TileContext(nc) as tc:
        with tc.tile_pool(name="persist", bufs=1) as pp:
            kT = pp.tile([128, DC, T], F16)      # k^T: [e, t], e-chunk major
            v_sb = pp.tile([128, TC, D], F16)    # v: [t, e], t-chunk major
            # zero-padded q^T: for head pair pr and query half sel, columns
            # [0:QH] hold head 2pr's q^T at partitions 0:64 (zeros below),
            # columns [QH:2QH] hold head 2pr+1's at partitions 64:128.
            # Keeps every scores matmul a full-128-partition K=128 matmul
            # (operands at base_partition 64 fail on hardware).
            qpad = pp.tile([128, DC, 2, 2 * QH], F16)
            aT = pp.tile([128, DC, QS], F16)     # attn^T: [d, q]

            nc.gpsimd.memset(qpad, 0.0)

            # ---------------- Phase 1: QKV projections ----------------
            with tc.tile_pool(name="ph1x", bufs=1) as p1x:
                xT = p1x.tile([128, DC, T], F16)
                nc.sync.dma_start(xT, chunked(xT_d))

                with (
                    tc.tile_pool(name="ph1q", bufs=1) as p1q,
                    tc.tile_pool(name="ppsq", bufs=4, space="PSUM") as ppsq,
                ):
                    xTq = p1q.tile([128, DC, QS], F16)
                    wq_sb = p1q.tile([128, DC, D], F16)
                    nc.sync.dma_start(xTq, chunked(xTq_d))
                    nc.sync.dma_start(wq_sb, chunked(wq_d))
                    # q^T[e, q] for this core's q-slice, written into the
                    # zero-padded layout (4 partition/half-sliced copies)
                    for ej in range(DC):
                        ps = ppsq.tile([128, 512], F32, tag="ps")
                        for jd in range(DC):
                            nc.tensor.matmul(
                                ps,
                                lhsT=wq_sb[:, jd, ej * 128:(ej + 1) * 128],
                                rhs=xTq[:, jd, :],
                                start=(jd == 0),
                                stop=(jd == DC - 1),
                            )
                        for sel in range(2):
                            nc.scalar.copy(
                                qpad[0:64, ej, sel, 0:QH],
                                ps[0:64, sel * QH:(sel + 1) * QH],
                            )
                            nc.scalar.copy(
                                qpad[64:128, ej, sel, QH:2 * QH],
                                ps[64:128, sel * QH:(sel + 1) * QH],
                            )

                with (
                    tc.tile_pool(name="ph1k", bufs=1) as p1k,
                    tc.tile_pool(name="ppsk", bufs=4, space="PSUM") as ppsk,
                ):
                    wk_sb = p1k.tile([128, DC, D], F16)
                    nc.sync.dma_start(wk_sb, chunked(wk_d))
                    # k^T[e, t] for the whole batch (tj outer: early key
                    # chunks complete first so attention can start sooner)
                    for tj in range(T // 512):
                        for ej in range(DC):
                            ps = ppsk.tile([128, 512], F32, tag="ps")
                            for jd in range(DC):
                                nc.tensor.matmul(
                                    ps,
                                    lhsT=wk_sb[:, jd, ej * 128:(ej + 1) * 128],
                                    rhs=xT[:, jd, tj * 512:(tj + 1) * 512],
                                    start=(jd == 0),
                                    stop=(jd == DC - 1),
                                )
                            nc.scalar.copy(
                                kT[:, ej, tj * 512:(tj + 1) * 512], ps
                            )

                with (
                    tc.tile_pool(name="ph1v", bufs=1) as p1v,
                    tc.tile_pool(name="ppsv", bufs=4, space="PSUM") as ppsv,
                ):
                    wv_sb = p1v.tile([128, DC, D], F16)
                    nc.sync.dma_start(wv_sb, chunked(wv_d))
                    # v[t, e] for the whole batch
                    for tj in range(TC):
                        for eh in range(2):
                            ps = ppsv.tile([128, 512], F32, tag="ps")
                            for jd in range(DC):
                                nc.tensor.matmul(
                                    ps,
                                    lhsT=xT[:, jd, tj * 128:(tj + 1) * 128],
                                    rhs=wv_sb[:, jd, eh * 512:(eh + 1) * 512],
                                    start=(jd == 0),
                                    stop=(jd == DC - 1),
                                )
                            nc.scalar.copy(
                                v_sb[:, tj, eh * 512:(eh + 1) * 512], ps
                            )

            # ---------------- Phase 2: attention ----------------
            with (
                tc.tile_pool(name="attps", bufs=2, space="PSUM") as aps,
                tc.tile_pool(name="scps", bufs=2, space="PSUM") as sps,
                tc.tile_pool(name="ework", bufs=2) as epool,
                tc.tile_pool(name="swork", bufs=2) as spool,
                tc.tile_pool(name="accp", bufs=1) as accpool,
            ):
                for qh in range(2):
                    acc = accpool.tile([128, DC, QH], F32, tag="acc")
                    for blk in range(NBLK):
                        Eb = epool.tile([128, KB, H, QH], F16, tag="Eb")
                        for kcl in range(KB):
                            kc = blk * KB + kcl
                            for g in range(4):  # 4 heads per PSUM tile
                                sc = sps.tile([128, 4 * QH], F32, tag="sc")
                                for i in range(2):  # head pairs 2g, 2g+1
                                    pr = 2 * g + i
                                    nc.tensor.matmul(
                                        sc[:, i * 2 * QH:(i + 1) * 2 * QH],
                                        lhsT=kT[:, pr,
                                                kc * 128:(kc + 1) * 128],
                                        rhs=qpad[:, pr, qh, :],
                                        start=True,
                                        stop=True,
                                    )
                                # fused PSUM evacuation + scale + exp
                                nc.scalar.activation(
                                    Eb[:, kcl, 4 * g:4 * g + 4, :],
                                    sc,
                                    EXP,
                                    scale=SCALE,
                                )
                            # S = sum over heads (log tree), R = 1/S, P = E*R
                            E = Eb[:, kcl]
                            tmp = spool.tile([128, H // 2, QH], F16, tag="tmp")
                            nc.vector.tensor_tensor(
                                tmp, E[:, 0:8], E[:, 8:16], ADD
                            )
                            nc.vector.tensor_tensor(
                                tmp[:, 0:4], tmp[:, 0:4], tmp[:, 4:8], ADD
                            )
                            nc.vector.tensor_tensor(
                                tmp[:, 0:2], tmp[:, 0:2], tmp[:, 2:4], ADD
                            )
                            nc.vector.tensor_tensor(
                                tmp[:, 0:1], tmp[:, 0:1], tmp[:, 1:2], ADD
                            )
                            r = spool.tile([128, 1, QH], F16, tag="r")
                            with nc.allow_low_precision(
                                reason="softmax denominator reciprocal in fp16"
                            ):
                                nc.vector.reciprocal(r, tmp[:, 0:1])
                            nc.vector.tensor_tensor(
                                E[:, 0:8], E[:, 0:8],
                                r.to_broadcast([128, 8, QH]), MULT
                            )
                            nc.gpsimd.tensor_tensor(
                                E[:, 8:16], E[:, 8:16],
                                r.to_broadcast([128, 8, QH]), MULT
                            )
                        # attn^T: 4 waves x 2 d-chunks; one accumulation
                        # group per full PSUM bank (128 partitions), two
                        # zero-padded per-head matmuls per key chunk. 2-bank
                        # wave tiles with bufs=2 so the next wave's matmuls
                        # overlap this wave's VectorE spill-add.
                        for w in range(4):
                            ps = aps.tile([128, 2, 2 * QH], F32, tag="wv")
                            for kcl in range(KB):
                                kc = blk * KB + kcl
                                for jdl in range(2):
                                    for par in range(2):
                                        h = 4 * w + 2 * jdl + par
                                        lo = par * 64
                                        nc.tensor.matmul(
                                            ps[lo:lo + 64, jdl, 0:QH],
                                            lhsT=v_sb[:, kc,
                                                      h * 64:(h + 1) * 64],
                                            rhs=Eb[:, kcl, h, :],
                                            start=(kcl == 0),
                                            stop=(kcl == KB - 1),
                                            skip_group_check=True,
                                        )
                            if blk == 0:
                                nc.vector.tensor_copy(
                                    acc[:, 2 * w:2 * w + 2, :], ps[:, :, 0:QH]
                                )
                            elif blk == NBLK - 1:
                                # final spill writes the fp16 attn^T tile
                                # directly (saves a ScalarE conversion pass)
                                nc.vector.tensor_tensor(
                                    aT[:, 2 * w:2 * w + 2,
                                       qh * QH:(qh + 1) * QH],
                                    ps[:, :, 0:QH],
                                    acc[:, 2 * w:2 * w + 2, :],
                                    ADD,
                                )
                            else:
                                nc.vector.tensor_tensor(
                                    acc[:, 2 * w:2 * w + 2, :],
                                    ps[:, :, 0:QH],
                                    acc[:, 2 * w:2 * w + 2, :],
                                    ADD,
                                )


            # ---------------- Phase 3: output projection ----------------
            out_ch = chunked(out_d)  # [128, QS//128, D]
            with (
                tc.tile_pool(name="prj", bufs=2, space="PSUM") as prj,
                tc.tile_pool(name="outp", bufs=2) as opool,
                tc.tile_pool(name="wpp", bufs=1) as wpp,
            ):
                wp_sb = wpp.tile([128, DC, D], F16)
                bi_sb = wpp.tile([128, D], F32)
                nc.sync.dma_start(wp_sb, chunked(wp_d))
                nc.sync.dma_start(bi_sb, bias_d)
                for qs in range(QS // 128):
                    for eh in range(2):
                        pm = prj.tile([128, 512], F32, tag="pm")
                        for jd in range(DC):
                            nc.tensor.matmul(
                                pm,
                                lhsT=aT[:, jd, qs * 128:(qs + 1) * 128],
                                rhs=wp_sb[:, jd, eh * 512:(eh + 1) * 512],
                                start=(jd == 0),
                                stop=(jd == DC - 1),
                            )
                        ot = opool.tile([128, 512], F32, tag="ot")
                        nc.vector.tensor_tensor(
                            ot, pm, bi_sb[:, eh * 512:(eh + 1) * 512], ADD
                        )
                        nc.sync.dma_start(
                            out_ch[:, qs, eh * 512:(eh + 1) * 512], ot
                        )

    nc.compile()
    return nc


def get_nc():
    global _CACHED_NC
    if _CACHED_NC is None:
        _CACHED_NC = _build_nc()
    return _CACHED_NC


def kernel(x, w_qkv, w_proj, b_proj, _trace=False, _tmpdir=None):
    x = np.asarray(x, dtype=np.float32)
    w_qkv = np.asarray(w_qkv, dtype=np.float32)
    w_proj = np.asarray(w_proj, dtype=np.float32)
    b_proj = np.asarray(b_proj, dtype=np.float32)

    # Host-side layout prep: transpose + fp16 casts + shard.
    xT = [np.ascontiguousarray(x[b].T).astype(np.float16) for b in range(B)]
    wq = np.ascontiguousarray(w_qkv[:, 0:D]).astype(np.float16)
    wk = np.ascontiguousarray(w_qkv[:, D:2 * D]).astype(np.float16)
    wv = np.ascontiguousarray(w_qkv[:, 2 * D:3 * D]).astype(np.float16)
    wp = w_proj.astype(np.float16)
    bias = np.ascontiguousarray(
        np.broadcast_to(b_proj, (128, D))
    ).astype(np.float32)

    in_maps = []
    for c in range(NCORES):
        b = c // (NCORES // B)
        qofs = (c % (NCORES // B)) * QS
        in_maps.append(
            {
                "xt": xT[b],
                "xtq": np.ascontiguousarray(xT[b][:, qofs:qofs + QS]),
                "wq": wq,
                "wk": wk,
                "wv": wv,
                "wp": wp,
                "bias": bias,
            }
        )

    nc = get_nc()
    res = bass_utils.run_bass_kernel_spmd(
        nc,
        in_maps,
        core_ids=list(range(NCORES)),
        trace=_trace,
        tmpdir=_tmpdir,
    )

    out = np.empty((B, T, D), dtype=np.float32)
    for c in range(NCORES):
        b = c // (NCORES // B)
        qofs = (c % (NCORES // B)) * QS
        out[b, qofs:qofs + QS] = res.results[c]["out"]
    if _trace:
        kernel._last_results = res
    return out


# revision 17
# speedup vs baseline: 10905.4494x; 1.0015x over previous
"""Trainium2 Bass kernel for nn_Attention_46995532153449.

Module: qkv = x @ w_qkv; per-head scores = q k^T * hd^-0.5; softmax over the
HEAD axis (axis=1); attn = probs @ v; out = attn @ w_proj + b_proj.

Shapes: B=2, T=2048, D=1024, H=16, HD=64.

Sharding: data-parallel over (batch, query-block). Core c handles batch
c // 4 and queries [(c % 4) * 512, (c % 4 + 1) * 512). The head-axis softmax
is local because every core holds all 16 heads for its query slice. Each
core recomputes K/V for its whole batch (replicated across the 4 cores of a
batch) so no collectives are needed.

Layout choices (all picked so that no on-chip transpose is ever required,
and so that every matmul is a full-128-partition matmul — operands at
base_partition 64 fail on this hardware):
  - host feeds x^T (fp16), so QKV projections produce q^T/k^T [e, t] with
    e on partitions (lhsT = W as-is, rhs = x^T) and v [t, e] (lhsT = x^T
    tile, rhs = Wv).
  - scores^T[k, q] per head via a zero-padded q^T (qpad): for head pair pr,
    columns [0:QH] hold head 2pr's q^T at partitions 0:64 (zeros at
    64:128) and columns [QH:2QH] hold head 2pr+1's at partitions 64:128.
    One K=128 matmul per pair (lhsT = k^T pair chunk, rhs = qpad) yields
    both heads' scores^T side by side. ScalarE evacuates the scores PSUM
    with a fused scale+exp into fp16 E tiles.
  - head-axis softmax: S = sum of the 16 E tiles (VectorE log-tree),
    R = 1/S (VectorE reciprocal), P = E * R broadcast — split across
    VectorE (heads 0:8) and GpSimd (heads 8:16) to balance engine load.
  - attn^T[d, q] = v^T P^T per head: lhsT = v tile [k, 64], rhs = P^T
    [k, q]; odd heads write output partitions 64:128 (col-tiled matmuls,
    concurrent with the even head's). Per-head PSUM accumulation groups
    share a bank partition-split (verified on HW: has_written clearing is
    per partition; the simulator's bank-granular group check is skipped
    via skip_group_check). Accumulated over KB=4 key-chunk blocks in
    PSUM, then spill-added into an SBUF fp32 accumulator on VectorE.
  - out[q, e]: lhsT = attn^T tile [d, q], rhs = w_proj [d, e]. Output is in
    natural [q, e] order for a contiguous DMA; bias added during PSUM
    evacuation.

Measured on the 8-core axon trn2 target: max rel err 6.7e-4 vs a float64
reference; cost-model timeline estimate ~406 us/core.
"""

import numpy as np

import concourse.bacc as bacc
import concourse.mybir as mybir
import concourse.tile as tile
from concourse import bass_utils

B, T, D, H = 2, 2048, 1024, 16
HD = D // H          # 64
SCALE = HD ** -0.5   # 0.125
NCORES = 8
QS = B * T // NCORES  # 512 queries per core
DC = D // 128         # 8 d/e chunks of 128
TC = T // 128         # 16 key chunks of 128
QH = QS // 2          # 256, query half (PSUM budget)
KB = 4                # key chunks per attention block
NBLK = TC // KB

F16 = mybir.dt.float16
F32 = mybir.dt.float32
ADD = mybir.AluOpType.add
MULT = mybir.AluOpType.mult
EXP = mybir.ActivationFunctionType.Exp

_CACHED_NC = None


def _build_nc():
    nc = bacc.Bacc(
        "TRN2", target_bir_lowering=False, debug=False, enable_asserts=False
    )

    xT_d = nc.dram_tensor("xt", [D, T], F16, kind="ExternalInput").ap()
    xTq_d = nc.dram_tensor("xtq", [D, QS], F16, kind="ExternalInput").ap()
    wq_d = nc.dram_tensor("wq", [D, D], F16, kind="ExternalInput").ap()
    wk_d = nc.dram_tensor("wk", [D, D], F16, kind="ExternalInput").ap()
    wv_d = nc.dram_tensor("wv", [D, D], F16, kind="ExternalInput").ap()
    wp_d = nc.dram_tensor("wp", [D, D], F16, kind="ExternalInput").ap()
    bias_d = nc.dram_tensor("bias", [128, D], F32, kind="ExternalInput").ap()
    out_d = nc.dram_tensor("out", [QS, D], F32, kind="ExternalOutput").ap()

    def chunked(ap):  # [(c p), f] -> [p, c, f]
        return ap.rearrange("(c p) f -> p c f", p=128)

    with tile.TileContext(nc) as tc:
        with tc.tile_pool(name="persist", bufs=1) as pp:
            kT = pp.tile([128, DC, T], F16)      # k^T: [e, t], e-chunk major
            v_sb = pp.tile([128, TC, D], F16)    # v: [t, e], t-chunk major
            # zero-padded q^T: for head pair pr and query half sel, columns
            # [0:QH] hold head 2pr's q^T at partitions 0:64 (zeros below),
            # columns [QH:2QH] hold head 2pr+1's at partitions 64:128.
            # Keeps every scores matmul a full-128-partition K=128 matmul
            # (operands at base_partition 64 fail on hardware).
            qpad = pp.tile([128, DC, 2, 2 * QH], F16)
            aT = pp.tile([128, DC, QS], F16)     # attn^T: [d, q]

            nc.gpsimd.memset(qpad, 0.0)

            # ---------------- Phase 1: QKV projections ----------------
            with tc.tile_pool(name="ph1x", bufs=1) as p1x:
                xT = p1x.tile([128, DC, T], F16)
                nc.sync.dma_start(xT, chunked(xT_d))

                with (
                    tc.tile_pool(name="ph1q", bufs=1) as p1q,
                    tc.tile_pool(name="ppsq", bufs=4, space="PSUM") as ppsq,
                ):
                    xTq = p1q.tile([128, DC, QS], F16)
                    wq_sb = p1q.tile([128, DC, D], F16)
                    nc.sync.dma_start(xTq, chunked(xTq_d))
                    nc.sync.dma_start(wq_sb, chunked(wq_d))
                    # q^T[e, q] for this core's q-slice, written into the
                    # zero-padded layout (4 partition/half-sliced copies)
                    for ej in range(DC):
                        ps = ppsq.tile([128, 512], F32, tag="ps")
                        for jd in range(DC):
                            nc.tensor.matmul(
                                ps,
                                lhsT=wq_sb[:, jd, ej * 128:(ej + 1) * 128],
                                rhs=xTq[:, jd, :],
                                start=(jd == 0),
                                stop=(jd == DC - 1),
                            )
                        for sel in range(2):
                            nc.scalar.copy(
                                qpad[0:64, ej, sel, 0:QH],
                                ps[0:64, sel * QH:(sel + 1) * QH],
                            )
                            nc.scalar.copy(
                                qpad[64:128, ej, sel, QH:2 * QH],
                                ps[64:128, sel * QH:(sel + 1) * QH],
                            )

                with (
                    tc.tile_pool(name="ph1k", bufs=1) as p1k,
                    tc.tile_pool(name="ppsk", bufs=4, space="PSUM") as ppsk,
                ):
                    wk_sb = p1k.tile([128, DC, D], F16)
                    nc.sync.dma_start(wk_sb, chunked(wk_d))
                    # k^T[e, t] for the whole batch (tj outer: early key
                    # chunks complete first so attention can start sooner)
                    for tj in range(T // 512):
                        for ej in range(DC):
                            ps = ppsk.tile([128, 512], F32, tag="ps")
                            for jd in range(DC):
                                nc.tensor.matmul(
                                    ps,
                                    lhsT=wk_sb[:, jd, ej * 128:(ej + 1) * 128],
                                    rhs=xT[:, jd, tj * 512:(tj + 1) * 512],
                                    start=(jd == 0),
                                    stop=(jd == DC - 1),
                                )
                            nc.scalar.copy(
                                kT[:, ej, tj * 512:(tj + 1) * 512], ps
                            )

                with (
                    tc.tile_pool(name="ph1v", bufs=1) as p1v,
                    tc.tile_pool(name="ppsv", bufs=4, space="PSUM") as ppsv,
                ):
                    wv_sb = p1v.tile([128, DC, D], F16)
                    nc.sync.dma_start(wv_sb, chunked(wv_d))
                    # v[t, e] for the whole batch
                    for tj in range(TC):
                        for eh in range(2):
                            ps = ppsv.tile([128, 512], F32, tag="ps")
                            for jd in range(DC):
                                nc.tensor.matmul(
                                    ps,
                                    lhsT=xT[:, jd, tj * 128:(tj + 1) * 128],
                                    rhs=wv_sb[:, jd, eh * 512:(eh + 1) * 512],
                                    start=(jd == 0),
                                    stop=(jd == DC - 1),
                                )
                            nc.vector.tensor_copy(
                                v_sb[:, tj, eh * 512:(eh + 1) * 512], ps
                            )

            # ---------------- Phase 2: attention ----------------
            with (
                tc.tile_pool(name="attps", bufs=2, space="PSUM") as aps,
                tc.tile_pool(name="scps", bufs=2, space="PSUM") as sps,
                tc.tile_pool(name="ework", bufs=2) as epool,
                tc.tile_pool(name="swork", bufs=2) as spool,
                tc.tile_pool(name="accp", bufs=1) as accpool,
            ):
                for qh in range(2):
                    acc = accpool.tile([128, DC, QH], F32, tag="acc")
                    for blk in range(NBLK):
                        Eb = epool.tile([128, KB, H, QH], F16, tag="Eb")
                        for kcl in range(KB):
                            kc = blk * KB + kcl
                            for g in range(4):  # 4 heads per PSUM tile
                                sc = sps.tile([128, 4 * QH], F32, tag="sc")
                                for i in range(2):  # head pairs 2g, 2g+1
                                    pr = 2 * g + i
                                    nc.tensor.matmul(
                                        sc[:, i * 2 * QH:(i + 1) * 2 * QH],
                                        lhsT=kT[:, pr,
                                                kc * 128:(kc + 1) * 128],
                                        rhs=qpad[:, pr, qh, :],
                                        start=True,
                                        stop=True,
                                    )
                                # fused PSUM evacuation + scale + exp
                                nc.scalar.activation(
                                    Eb[:, kcl, 4 * g:4 * g + 4, :],
                                    sc,
                                    EXP,
                                    scale=SCALE,
                                )
                            # S = sum over heads (log tree), R = 1/S, P = E*R
                            E = Eb[:, kcl]
                            tmp = spool.tile([128, H // 2, QH], F16, tag="tmp")
                            nc.vector.tensor_tensor(
                                tmp, E[:, 0:8], E[:, 8:16], ADD
                            )
                            nc.vector.tensor_tensor(
                                tmp[:, 0:4], tmp[:, 0:4], tmp[:, 4:8], ADD
                            )
                            nc.vector.tensor_tensor(
                                tmp[:, 0:2], tmp[:, 0:2], tmp[:, 2:4], ADD
                            )
                            nc.vector.tensor_tensor(
                                tmp[:, 0:1], tmp[:, 0:1], tmp[:, 1:2], ADD
                            )
                            r = spool.tile([128, 1, QH], F16, tag="r")
                            with nc.allow_low_precision(
                                reason="softmax denominator reciprocal in fp16"
                            ):
                                nc.vector.reciprocal(r, tmp[:, 0:1])
                            nc.vector.tensor_tensor(
                                E[:, 0:8], E[:, 0:8],
                                r.to_broadcast([128, 8, QH]), MULT
                            )
                            nc.gpsimd.tensor_tensor(
                                E[:, 8:16], E[:, 8:16],
                                r.to_broadcast([128, 8, QH]), MULT
                            )
                        # attn^T: 4 waves x 2 d-chunks; one accumulation
                        # group per full PSUM bank (128 partitions), two
                        # zero-padded per-head matmuls per key chunk. 2-bank
                        # wave tiles with bufs=2 so the next wave's matmuls
                        # overlap this wave's VectorE spill-add.
                        for w in range(4):
                            ps = aps.tile([128, 2, 2 * QH], F32, tag="wv")
                            for kcl in range(KB):
                                kc = blk * KB + kcl
                                for jdl in range(2):
                                    for par in range(2):
                                        h = 4 * w + 2 * jdl + par
                                        lo = par * 64
                                        nc.tensor.matmul(
                                            ps[lo:lo + 64, jdl, 0:QH],
                                            lhsT=v_sb[:, kc,
                                                      h * 64:(h + 1) * 64],
                                            rhs=Eb[:, kcl, h, :],
                                            start=(kcl == 0),
                                            stop=(kcl == KB - 1),
                                            skip_group_check=True,
                                        )
                            if blk == 0:
                                nc.vector.tensor_copy(
                                    acc[:, 2 * w:2 * w + 2, :], ps[:, :, 0:QH]
                                )
                            elif blk == NBLK - 1:
                                # final spill writes the fp16 attn^T tile
                                # directly (saves a ScalarE conversion pass)
                                nc.vector.tensor_tensor(
                                    aT[:, 2 * w:2 * w + 2,
                                       qh * QH:(qh + 1) * QH],
                                    ps[:, :, 0:QH],
                                    acc[:, 2 * w:2 * w + 2, :],
                                    ADD,
                                )
                            else:
                                nc.vector.tensor_tensor(
                                    acc[:, 2 * w:2 * w + 2, :],
                                    ps[:, :, 0:QH],
                                    acc[:, 2 * w:2 * w + 2, :],
                                    ADD,
                                )


            # ---------------- Phase 3: output projection ----------------
            out_ch = chunked(out_d)  # [128, QS//128, D]
            with (
                tc.tile_pool(name="prj", bufs=2, space="PSUM") as prj,
                tc.tile_pool(name="outp", bufs=2) as opool,
                tc.tile_pool(name="wpp", bufs=1) as wpp,
            ):
                wp_sb = wpp.tile([128, DC, D], F16)
                bi_sb = wpp.tile([128, D], F32)
                nc.sync.dma_start(wp_sb, chunked(wp_d))
                nc.sync.dma_start(bi_sb, bias_d)
                for qs in range(QS // 128):
                    for eh in range(2):
                        pm = prj.tile([128, 512], F32, tag="pm")
                        for jd in range(DC):
                            nc.tensor.matmul(
                                pm,
                                lhsT=aT[:, jd, qs * 128:(qs + 1) * 128],
                                rhs=wp_sb[:, jd, eh * 512:(eh + 1) * 512],
                                start=(jd == 0),
                                stop=(jd == DC - 1),
                            )
                        ot = opool.tile([128, 512], F32, tag="ot")
                        nc.vector.tensor_tensor(
                            ot, pm, bi_sb[:, eh * 512:(eh + 1) * 512], ADD
                        )
                        nc.sync.dma_start(
                            out_ch[:, qs, eh * 512:(eh + 1) * 512], ot
                        )

    nc.compile()
    return nc


def get_nc():
    global _CACHED_NC
    if _CACHED_NC is None:
        _CACHED_NC = _build_nc()
    return _CACHED_NC


def kernel(x, w_qkv, w_proj, b_proj, _trace=False, _tmpdir=None):
    x = np.asarray(x, dtype=np.float32)
    w_qkv = np.asarray(w_qkv, dtype=np.float32)
    w_proj = np.asarray(w_proj, dtype=np.float32)
    b_proj = np.asarray(b_proj, dtype=np.float32)

    # Host-side layout prep: transpose + fp16 casts + shard.
    xT = [np.ascontiguousarray(x[b].T).astype(np.float16) for b in range(B)]
    wq = np.ascontiguousarray(w_qkv[:, 0:D]).astype(np.float16)
    wk = np.ascontiguousarray(w_qkv[:, D:2 * D]).astype(np.float16)
    wv = np.ascontiguousarray(w_qkv[:, 2 * D:3 * D]).astype(np.float16)
    wp = w_proj.astype(np.float16)
    bias = np.ascontiguousarray(
        np.broadcast_to(b_proj, (128, D))
    ).astype(np.float32)

    in_maps = []
    for c in range(NCORES):
        b = c // (NCORES // B)
        qofs = (c % (NCORES // B)) * QS
        in_maps.append(
            {
                "xt": xT[b],
                "xtq": np.ascontiguousarray(xT[b][:, qofs:qofs + QS]),
                "wq": wq,
                "wk": wk,
                "wv": wv,
                "wp": wp,
                "bias": bias,
            }
        )

    nc = get_nc()
    res = bass_utils.run_bass_kernel_spmd(
        nc,
        in_maps,
        core_ids=list(range(NCORES)),
        trace=_trace,
        tmpdir=_tmpdir,
    )

    out = np.empty((B, T, D), dtype=np.float32)
    for c in range(NCORES):
        b = c // (NCORES // B)
        qofs = (c % (NCORES // B)) * QS
        out[b, qofs:qofs + QS] = res.results[c]["out"]
    if _trace:
        kernel._last_results = res
    return out


# revision 18
# speedup vs baseline: 11197.0029x; 1.0267x over previous
"""Trainium2 Bass kernel for nn_Attention_46995532153449.

Module: qkv = x @ w_qkv; per-head scores = q k^T * hd^-0.5; softmax over the
HEAD axis (axis=1); attn = probs @ v; out = attn @ w_proj + b_proj.

Shapes: B=2, T=2048, D=1024, H=16, HD=64.

Sharding: data-parallel over (batch, query-block). Core c handles batch
c // 4 and queries [(c % 4) * 512, (c % 4 + 1) * 512). The head-axis softmax
is local because every core holds all 16 heads for its query slice. Each
core recomputes K/V for its whole batch (replicated across the 4 cores of a
batch) so no collectives are needed.

Layout choices (all picked so that no on-chip transpose is ever required,
and so that every matmul is a full-128-partition matmul — operands at
base_partition 64 fail on this hardware):
  - host feeds x^T (fp16), so QKV projections produce q^T/k^T [e, t] with
    e on partitions (lhsT = W as-is, rhs = x^T) and v [t, e] (lhsT = x^T
    tile, rhs = Wv).
  - scores^T[k, q] per head via a zero-padded q^T (qpad): for head pair pr,
    columns [0:QH] hold head 2pr's q^T at partitions 0:64 (zeros at
    64:128) and columns [QH:2QH] hold head 2pr+1's at partitions 64:128.
    One K=128 matmul per pair (lhsT = k^T pair chunk, rhs = qpad) yields
    both heads' scores^T side by side. ScalarE evacuates the scores PSUM
    with a fused scale+exp into fp16 E tiles.
  - head-axis softmax: S = sum of the 16 E tiles (VectorE log-tree),
    R = 1/S (VectorE reciprocal), P = E * R broadcast — split across
    VectorE (heads 0:8) and GpSimd (heads 8:16) to balance engine load.
  - attn^T[d, q] = v^T P^T per head: lhsT = v tile [k, 64], rhs = P^T
    [k, q]; odd heads write output partitions 64:128 (col-tiled matmuls,
    concurrent with the even head's). Per-head PSUM accumulation groups
    share a bank partition-split (verified on HW: has_written clearing is
    per partition; the simulator's bank-granular group check is skipped
    via skip_group_check). Accumulated over KB=4 key-chunk blocks in
    PSUM, then spill-added into an SBUF fp32 accumulator on VectorE.
  - out[q, e]: lhsT = attn^T tile [d, q], rhs = w_proj [d, e]. Output is in
    natural [q, e] order for a contiguous DMA; bias added during PSUM
    evacuation.

Measured on the 8-core axon trn2 target: max rel err 6.7e-4 vs a float64
reference; cost-model timeline estimate ~406 us/core.
"""

import numpy as np

import concourse.bacc as bacc
import concourse.mybir as mybir
import concourse.tile as tile
from concourse import bass_utils

B, T, D, H = 2, 2048, 1024, 16
HD = D // H          # 64
SCALE = HD ** -0.5   # 0.125
NCORES = 8
QS = B * T // NCORES  # 512 queries per core
DC = D // 128         # 8 d/e chunks of 128
TC = T // 128         # 16 key chunks of 128
QH = QS // 2          # 256, query half (PSUM budget)
KB = 4                # key chunks per attention block
NBLK = TC // KB

F16 = mybir.dt.float16
F32 = mybir.dt.float32
ADD = mybir.AluOpType.add
MULT = mybir.AluOpType.mult
EXP = mybir.ActivationFunctionType.Exp

_CACHED_NC = None


def _build_nc():
    nc = bacc.Bacc(
        "TRN2", target_bir_lowering=False, debug=False, enable_asserts=False
    )

    xT_d = nc.dram_tensor("xt", [D, T], F16, kind="ExternalInput").ap()
    xTq_d = nc.dram_tensor("xtq", [D, QS], F16, kind="ExternalInput").ap()
    wq_d = nc.dram_tensor("wq", [D, D], F16, kind="ExternalInput").ap()
    wk_d = nc.dram_tensor("wk", [D, D], F16, kind="ExternalInput").ap()
    wv_d = nc.dram_tensor("wv", [D, D], F16, kind="ExternalInput").ap()
    wp_d = nc.dram_tensor("wp", [D, D], F16, kind="ExternalInput").ap()
    bias_d = nc.dram_tensor("bias", [128, D], F32, kind="ExternalInput").ap()
    out_d = nc.dram_tensor("out", [QS, D], F32, kind="ExternalOutput").ap()

    def chunked(ap):  # [(c p), f] -> [p, c, f]
        return ap.rearrange("(c p) f -> p c f", p=128)

    with tile.TileContext(nc) as tc:
        with tc.tile_pool(name="persist", bufs=1) as pp:
            kT = pp.tile([128, DC, T], F16)      # k^T: [e, t], e-chunk major
            v_sb = pp.tile([128, TC, D], F16)    # v: [t, e], t-chunk major
            # zero-padded q^T: for head pair pr and query half sel, columns
            # [0:QH] hold head 2pr's q^T at partitions 0:64 (zeros below),
            # columns [QH:2QH] hold head 2pr+1's at partitions 64:128.
            # Keeps every scores matmul a full-128-partition K=128 matmul
            # (operands at base_partition 64 fail on hardware).
            qpad = pp.tile([128, DC, 2, 2 * QH], F16)
            aT = pp.tile([128, DC, QS], F16)     # attn^T: [d, q]
            wp_sb = pp.tile([128, DC, D], F16)
            bi_sb = pp.tile([128, D], F32)

            nc.gpsimd.memset(qpad, 0.0)
            nc.sync.dma_start(wp_sb, chunked(wp_d))
            nc.sync.dma_start(bi_sb, bias_d)

            # ---------------- Phase 1: QKV projections ----------------
            with tc.tile_pool(name="ph1x", bufs=1) as p1x:
                xT = p1x.tile([128, DC, T], F16)
                nc.sync.dma_start(xT, chunked(xT_d))

                with (
                    tc.tile_pool(name="ph1q", bufs=1) as p1q,
                    tc.tile_pool(name="ppsq", bufs=4, space="PSUM") as ppsq,
                ):
                    xTq = p1q.tile([128, DC, QS], F16)
                    wq_sb = p1q.tile([128, DC, D], F16)
                    nc.sync.dma_start(xTq, chunked(xTq_d))
                    nc.sync.dma_start(wq_sb, chunked(wq_d))
                    # q^T[e, q] for this core's q-slice, written into the
                    # zero-padded layout (4 partition/half-sliced copies)
                    for ej in range(DC):
                        ps = ppsq.tile([128, 512], F32, tag="ps")
                        for jd in range(DC):
                            nc.tensor.matmul(
                                ps,
                                lhsT=wq_sb[:, jd, ej * 128:(ej + 1) * 128],
                                rhs=xTq[:, jd, :],
                                start=(jd == 0),
                                stop=(jd == DC - 1),
                            )
                        for sel in range(2):
                            nc.scalar.copy(
                                qpad[0:64, ej, sel, 0:QH],
                                ps[0:64, sel * QH:(sel + 1) * QH],
                            )
                            nc.scalar.copy(
                                qpad[64:128, ej, sel, QH:2 * QH],
                                ps[64:128, sel * QH:(sel + 1) * QH],
                            )

                with (
                    tc.tile_pool(name="ph1k", bufs=1) as p1k,
                    tc.tile_pool(name="ppsk", bufs=4, space="PSUM") as ppsk,
                ):
                    wk_sb = p1k.tile([128, DC, D], F16)
                    nc.sync.dma_start(wk_sb, chunked(wk_d))
                    # k^T[e, t] for the whole batch (tj outer: early key
                    # chunks complete first so attention can start sooner)
                    for tj in range(T // 512):
                        for ej in range(DC):
                            ps = ppsk.tile([128, 512], F32, tag="ps")
                            for jd in range(DC):
                                nc.tensor.matmul(
                                    ps,
                                    lhsT=wk_sb[:, jd, ej * 128:(ej + 1) * 128],
                                    rhs=xT[:, jd, tj * 512:(tj + 1) * 512],
                                    start=(jd == 0),
                                    stop=(jd == DC - 1),
                                )
                            nc.scalar.copy(
                                kT[:, ej, tj * 512:(tj + 1) * 512], ps
                            )

                with (
                    tc.tile_pool(name="ph1v", bufs=1) as p1v,
                    tc.tile_pool(name="ppsv", bufs=4, space="PSUM") as ppsv,
                ):
                    wv_sb = p1v.tile([128, DC, D], F16)
                    nc.sync.dma_start(wv_sb, chunked(wv_d))
                    # v[t, e] for the whole batch
                    for tj in range(TC):
                        for eh in range(2):
                            ps = ppsv.tile([128, 512], F32, tag="ps")
                            for jd in range(DC):
                                nc.tensor.matmul(
                                    ps,
                                    lhsT=xT[:, jd, tj * 128:(tj + 1) * 128],
                                    rhs=wv_sb[:, jd, eh * 512:(eh + 1) * 512],
                                    start=(jd == 0),
                                    stop=(jd == DC - 1),
                                )
                            nc.vector.tensor_copy(
                                v_sb[:, tj, eh * 512:(eh + 1) * 512], ps
                            )

            # ---------------- Phase 2: attention ----------------
            with (
                tc.tile_pool(name="attps", bufs=2, space="PSUM") as aps,
                tc.tile_pool(name="scps", bufs=2, space="PSUM") as sps,
                tc.tile_pool(name="ework", bufs=2) as epool,
                tc.tile_pool(name="swork", bufs=2) as spool,
                tc.tile_pool(name="accp", bufs=1) as accpool,
            ):
                for qh in range(2):
                    acc = accpool.tile([128, DC, QH], F32, tag="acc")
                    for blk in range(NBLK):
                        Eb = epool.tile([128, KB, H, QH], F16, tag="Eb")
                        for kcl in range(KB):
                            kc = blk * KB + kcl
                            for g in range(4):  # 4 heads per PSUM tile
                                sc = sps.tile([128, 4 * QH], F32, tag="sc")
                                for i in range(2):  # head pairs 2g, 2g+1
                                    pr = 2 * g + i
                                    nc.tensor.matmul(
                                        sc[:, i * 2 * QH:(i + 1) * 2 * QH],
                                        lhsT=kT[:, pr,
                                                kc * 128:(kc + 1) * 128],
                                        rhs=qpad[:, pr, qh, :],
                                        start=True,
                                        stop=True,
                                    )
                                # fused PSUM evacuation + scale + exp
                                nc.scalar.activation(
                                    Eb[:, kcl, 4 * g:4 * g + 4, :],
                                    sc,
                                    EXP,
                                    scale=SCALE,
                                )
                            # S = sum over heads (log tree), R = 1/S, P = E*R
                            E = Eb[:, kcl]
                            tmp = spool.tile([128, H // 2, QH], F16, tag="tmp")
                            nc.vector.tensor_tensor(
                                tmp, E[:, 0:8], E[:, 8:16], ADD
                            )
                            nc.vector.tensor_tensor(
                                tmp[:, 0:4], tmp[:, 0:4], tmp[:, 4:8], ADD
                            )
                            nc.vector.tensor_tensor(
                                tmp[:, 0:2], tmp[:, 0:2], tmp[:, 2:4], ADD
                            )
                            nc.vector.tensor_tensor(
                                tmp[:, 0:1], tmp[:, 0:1], tmp[:, 1:2], ADD
                            )
                            r = spool.tile([128, 1, QH], F16, tag="r")
                            with nc.allow_low_precision(
                                reason="softmax denominator reciprocal in fp16"
                            ):
                                nc.vector.reciprocal(r, tmp[:, 0:1])
                            nc.vector.tensor_tensor(
                                E[:, 0:8], E[:, 0:8],
                                r.to_broadcast([128, 8, QH]), MULT
                            )
                            nc.gpsimd.tensor_tensor(
                                E[:, 8:16], E[:, 8:16],
                                r.to_broadcast([128, 8, QH]), MULT
                            )
                        # attn^T: 4 waves x 2 d-chunks; one accumulation
                        # group per full PSUM bank (128 partitions), two
                        # zero-padded per-head matmuls per key chunk. 2-bank
                        # wave tiles with bufs=2 so the next wave's matmuls
                        # overlap this wave's VectorE spill-add.
                        for w in range(4):
                            ps = aps.tile([128, 2, 2 * QH], F32, tag="wv")
                            for kcl in range(KB):
                                kc = blk * KB + kcl
                                for jdl in range(2):
                                    for par in range(2):
                                        h = 4 * w + 2 * jdl + par
                                        lo = par * 64
                                        nc.tensor.matmul(
                                            ps[lo:lo + 64, jdl, 0:QH],
                                            lhsT=v_sb[:, kc,
                                                      h * 64:(h + 1) * 64],
                                            rhs=Eb[:, kcl, h, :],
                                            start=(kcl == 0),
                                            stop=(kcl == KB - 1),
                                            skip_group_check=True,
                                        )
                            if blk == 0:
                                nc.vector.tensor_copy(
                                    acc[:, 2 * w:2 * w + 2, :], ps[:, :, 0:QH]
                                )
                            elif blk == NBLK - 1:
                                # final spill writes the fp16 attn^T tile
                                # directly (saves a ScalarE conversion pass)
                                nc.vector.tensor_tensor(
                                    aT[:, 2 * w:2 * w + 2,
                                       qh * QH:(qh + 1) * QH],
                                    ps[:, :, 0:QH],
                                    acc[:, 2 * w:2 * w + 2, :],
                                    ADD,
                                )
                            else:
                                nc.vector.tensor_tensor(
                                    acc[:, 2 * w:2 * w + 2, :],
                                    ps[:, :, 0:QH],
                                    acc[:, 2 * w:2 * w + 2, :],
                                    ADD,
                                )


            # ---------------- Phase 3: output projection ----------------
            out_ch = chunked(out_d)  # [128, QS//128, D]
            with (
                tc.tile_pool(name="prj", bufs=2, space="PSUM") as prj,
                tc.tile_pool(name="outp", bufs=2) as opool,
            ):
                for qs in range(QS // 128):
                    for eh in range(2):
                        pm = prj.tile([128, 512], F32, tag="pm")
                        for jd in range(DC):
                            nc.tensor.matmul(
                                pm,
                                lhsT=aT[:, jd, qs * 128:(qs + 1) * 128],
                                rhs=wp_sb[:, jd, eh * 512:(eh + 1) * 512],
                                start=(jd == 0),
                                stop=(jd == DC - 1),
                            )
                        ot = opool.tile([128, 512], F32, tag="ot")
                        nc.vector.tensor_tensor(
                            ot, pm, bi_sb[:, eh * 512:(eh + 1) * 512], ADD
                        )
                        nc.sync.dma_start(
                            out_ch[:, qs, eh * 512:(eh + 1) * 512], ot
                        )

    nc.compile()
    return nc


def get_nc():
    global _CACHED_NC
    if _CACHED_NC is None:
        _CACHED_NC = _build_nc()
    return _CACHED_NC


def kernel(x, w_qkv, w_proj, b_proj, _trace=False, _tmpdir=None):
    x = np.asarray(x, dtype=np.float32)
    w_qkv = np.asarray(w_qkv, dtype=np.float32)
    w_proj = np.asarray(w_proj, dtype=np.float32)
    b_proj = np.asarray(b_proj, dtype=np.float32)

    # Host-side layout prep: transpose + fp16 casts + shard.
    xT = [np.ascontiguousarray(x[b].T).astype(np.float16) for b in range(B)]
    wq = np.ascontiguousarray(w_qkv[:, 0:D]).astype(np.float16)
    wk = np.ascontiguousarray(w_qkv[:, D:2 * D]).astype(np.float16)
    wv = np.ascontiguousarray(w_qkv[:, 2 * D:3 * D]).astype(np.float16)
    wp = w_proj.astype(np.float16)
    bias = np.ascontiguousarray(
        np.broadcast_to(b_proj, (128, D))
    ).astype(np.float32)

    in_maps = []
    for c in range(NCORES):
        b = c // (NCORES // B)
        qofs = (c % (NCORES // B)) * QS
        in_maps.append(
            {
                "xt": xT[b],
                "xtq": np.ascontiguousarray(xT[b][:, qofs:qofs + QS]),
                "wq": wq,
                "wk": wk,
                "wv": wv,
                "wp": wp,
                "bias": bias,
            }
        )

    nc = get_nc()
    res = bass_utils.run_bass_kernel_spmd(
        nc,
        in_maps,
        core_ids=list(range(NCORES)),
        trace=_trace,
        tmpdir=_tmpdir,
    )

    out = np.empty((B, T, D), dtype=np.float32)
    for c in range(NCORES):
        b = c // (NCORES // B)
        qofs = (c % (NCORES // B)) * QS
        out[b, qofs:qofs + QS] = res.results[c]["out"]
    if _trace:
        kernel._last_results = res
    return out


# revision 20
# speedup vs baseline: 11538.4489x; 1.0305x over previous
"""Trainium2 Bass kernel for nn_Attention_46995532153449.

Module: qkv = x @ w_qkv; per-head scores = q k^T * hd^-0.5; softmax over the
HEAD axis (axis=1); attn = probs @ v; out = attn @ w_proj + b_proj.

Shapes: B=2, T=2048, D=1024, H=16, HD=64.

Sharding: data-parallel over (batch, query-block). Core c handles batch
c // 4 and queries [(c % 4) * 512, (c % 4 + 1) * 512). The head-axis softmax
is local because every core holds all 16 heads for its query slice. Each
core recomputes K/V for its whole batch (replicated across the 4 cores of a
batch) so no collectives are needed.

Layout choices (all picked so that no on-chip transpose is ever required,
and so that every matmul is a full-128-partition matmul — operands at
base_partition 64 fail on this hardware):
  - host feeds x^T (fp16), so QKV projections produce q^T/k^T [e, t] with
    e on partitions (lhsT = W as-is, rhs = x^T) and v [t, e] (lhsT = x^T
    tile, rhs = Wv).
  - scores^T[k, q] per head via a zero-padded q^T (qpad): for head pair pr,
    columns [0:QH] hold head 2pr's q^T at partitions 0:64 (zeros at
    64:128) and columns [QH:2QH] hold head 2pr+1's at partitions 64:128.
    One K=128 matmul per pair (lhsT = k^T pair chunk, rhs = qpad) yields
    both heads' scores^T side by side. ScalarE evacuates the scores PSUM
    with a fused scale+exp into fp16 E tiles.
  - head-axis softmax: S = sum of the 16 E tiles (VectorE log-tree),
    R = 1/S (VectorE reciprocal), P = E * R broadcast — split across
    VectorE (heads 0:8) and GpSimd (heads 8:16) to balance engine load.
  - attn^T[d, q] = v^T P^T per head: lhsT = v tile [k, 64], rhs = P^T
    [k, q]; odd heads write output partitions 64:128 (col-tiled matmuls,
    concurrent with the even head's). Per-head PSUM accumulation groups
    share a bank partition-split (verified on HW: has_written clearing is
    per partition; the simulator's bank-granular group check is skipped
    via skip_group_check). Accumulated over KB=4 key-chunk blocks in
    PSUM, then spill-added into an SBUF fp32 accumulator on VectorE.
  - out[q, e]: lhsT = attn^T tile [d, q], rhs = w_proj [d, e]. Output is in
    natural [q, e] order for a contiguous DMA; bias added during PSUM
    evacuation.

Measured on the 8-core axon trn2 target: max rel err 6.7e-4 vs a float64
reference; cost-model timeline estimate ~394 us/core.
"""

import numpy as np

import concourse.bacc as bacc
import concourse.mybir as mybir
import concourse.tile as tile
from concourse import bass_utils

B, T, D, H = 2, 2048, 1024, 16
HD = D // H          # 64
SCALE = HD ** -0.5   # 0.125
NCORES = 8
QS = B * T // NCORES  # 512 queries per core
DC = D // 128         # 8 d/e chunks of 128
TC = T // 128         # 16 key chunks of 128
QH = QS // 2          # 256, query half (PSUM budget)
KB = 4                # key chunks per attention block
NBLK = TC // KB

F16 = mybir.dt.float16
F32 = mybir.dt.float32
ADD = mybir.AluOpType.add
MULT = mybir.AluOpType.mult
EXP = mybir.ActivationFunctionType.Exp

_CACHED_NC = None


def _build_nc():
    nc = bacc.Bacc(
        "TRN2", target_bir_lowering=False, debug=False, enable_asserts=False
    )

    xT_d = nc.dram_tensor("xt", [D, T], F16, kind="ExternalInput").ap()
    xTq_d = nc.dram_tensor("xtq", [D, QS], F16, kind="ExternalInput").ap()
    wq_d = nc.dram_tensor("wq", [D, D], F16, kind="ExternalInput").ap()
    wk_d = nc.dram_tensor("wk", [D, D], F16, kind="ExternalInput").ap()
    wv_d = nc.dram_tensor("wv", [D, D], F16, kind="ExternalInput").ap()
    wp_d = nc.dram_tensor("wp", [D, D], F16, kind="ExternalInput").ap()
    bias_d = nc.dram_tensor("bias", [128, D], F32, kind="ExternalInput").ap()
    out_d = nc.dram_tensor("out", [QS, D], F32, kind="ExternalOutput").ap()

    def chunked(ap):  # [(c p), f] -> [p, c, f]
        return ap.rearrange("(c p) f -> p c f", p=128)

    with tile.TileContext(nc) as tc:
        with tc.tile_pool(name="persist", bufs=1) as pp:
            kT = pp.tile([128, DC, T], F16)      # k^T: [e, t], e-chunk major
            v_sb = pp.tile([128, TC, D], F16)    # v: [t, e], t-chunk major
            # zero-padded q^T: for head pair pr and query half sel, columns
            # [0:QH] hold head 2pr's q^T at partitions 0:64 (zeros below),
            # columns [QH:2QH] hold head 2pr+1's at partitions 64:128.
            # Keeps every scores matmul a full-128-partition K=128 matmul
            # (operands at base_partition 64 fail on hardware).
            qpad = pp.tile([128, DC, 2, 2 * QH], F16)
            aT = pp.tile([128, DC, QS], F16)     # attn^T: [d, q]
            wp_sb = pp.tile([128, DC, D], F16)
            bi_sb = pp.tile([128, D], F32)

            nc.gpsimd.memset(qpad, 0.0)
            nc.sync.dma_start(wp_sb, chunked(wp_d))
            nc.sync.dma_start(bi_sb, bias_d)

            # ---------------- Phase 1: QKV projections ----------------
            with tc.tile_pool(name="ph1x", bufs=1) as p1x:
                xT = p1x.tile([128, DC, T], F16)

                with (
                    tc.tile_pool(name="ph1q", bufs=1) as p1q,
                    tc.tile_pool(name="ppsq", bufs=4, space="PSUM") as ppsq,
                ):
                    xTq = p1q.tile([128, DC, QS], F16)
                    wq_sb = p1q.tile([128, DC, D], F16)
                    # Q's inputs first: the DMA ring is FIFO and these gate
                    # the kernel's first matmuls; the big x^T transfer follows
                    nc.sync.dma_start(xTq, chunked(xTq_d))
                    nc.sync.dma_start(wq_sb, chunked(wq_d))
                    nc.sync.dma_start(xT, chunked(xT_d))
                    # q^T[e, q] for this core's q-slice, written into the
                    # zero-padded layout (4 partition/half-sliced copies)
                    for ej in range(DC):
                        ps = ppsq.tile([128, 512], F32, tag="ps")
                        for jd in range(DC):
                            nc.tensor.matmul(
                                ps,
                                lhsT=wq_sb[:, jd, ej * 128:(ej + 1) * 128],
                                rhs=xTq[:, jd, :],
                                start=(jd == 0),
                                stop=(jd == DC - 1),
                            )
                        for sel in range(2):
                            nc.scalar.copy(
                                qpad[0:64, ej, sel, 0:QH],
                                ps[0:64, sel * QH:(sel + 1) * QH],
                            )
                            nc.scalar.copy(
                                qpad[64:128, ej, sel, QH:2 * QH],
                                ps[64:128, sel * QH:(sel + 1) * QH],
                            )

                with (
                    tc.tile_pool(name="ph1k", bufs=1) as p1k,
                    tc.tile_pool(name="ppsk", bufs=4, space="PSUM") as ppsk,
                ):
                    wk_sb = p1k.tile([128, DC, D], F16)
                    nc.sync.dma_start(wk_sb, chunked(wk_d))
                    # k^T[e, t] for the whole batch (tj outer: early key
                    # chunks complete first so attention can start sooner)
                    for tj in range(T // 512):
                        for ej in range(DC):
                            ps = ppsk.tile([128, 512], F32, tag="ps")
                            for jd in range(DC):
                                nc.tensor.matmul(
                                    ps,
                                    lhsT=wk_sb[:, jd, ej * 128:(ej + 1) * 128],
                                    rhs=xT[:, jd, tj * 512:(tj + 1) * 512],
                                    start=(jd == 0),
                                    stop=(jd == DC - 1),
                                )
                            nc.scalar.copy(
                                kT[:, ej, tj * 512:(tj + 1) * 512], ps
                            )

                with (
                    tc.tile_pool(name="ph1v", bufs=1) as p1v,
                    tc.tile_pool(name="ppsv", bufs=4, space="PSUM") as ppsv,
                ):
                    wv_sb = p1v.tile([128, DC, D], F16)
                    nc.sync.dma_start(wv_sb, chunked(wv_d))
                    # v[t, e] for the whole batch
                    for tj in range(TC):
                        for eh in range(2):
                            ps = ppsv.tile([128, 512], F32, tag="ps")
                            for jd in range(DC):
                                nc.tensor.matmul(
                                    ps,
                                    lhsT=xT[:, jd, tj * 128:(tj + 1) * 128],
                                    rhs=wv_sb[:, jd, eh * 512:(eh + 1) * 512],
                                    start=(jd == 0),
                                    stop=(jd == DC - 1),
                                )
                            nc.vector.tensor_copy(
                                v_sb[:, tj, eh * 512:(eh + 1) * 512], ps
                            )

            # ---------------- Phase 2: attention ----------------
            with (
                tc.tile_pool(name="attps", bufs=2, space="PSUM") as aps,
                tc.tile_pool(name="scps", bufs=2, space="PSUM") as sps,
                tc.tile_pool(name="ework", bufs=2) as epool,
                tc.tile_pool(name="swork", bufs=2) as spool,
                tc.tile_pool(name="accp", bufs=1) as accpool,
            ):
                for qh in range(2):
                    acc = accpool.tile([128, DC, QH], F32, tag="acc")
                    for blk in range(NBLK):
                        Eb = epool.tile([128, KB, H, QH], F16, tag="Eb")
                        for kcl in range(KB):
                            kc = blk * KB + kcl
                            for g in range(4):  # 4 heads per PSUM tile
                                sc = sps.tile([128, 4 * QH], F32, tag="sc")
                                for i in range(2):  # head pairs 2g, 2g+1
                                    pr = 2 * g + i
                                    nc.tensor.matmul(
                                        sc[:, i * 2 * QH:(i + 1) * 2 * QH],
                                        lhsT=kT[:, pr,
                                                kc * 128:(kc + 1) * 128],
                                        rhs=qpad[:, pr, qh, :],
                                        start=True,
                                        stop=True,
                                    )
                                # fused PSUM evacuation + scale + exp
                                nc.scalar.activation(
                                    Eb[:, kcl, 4 * g:4 * g + 4, :],
                                    sc,
                                    EXP,
                                    scale=SCALE,
                                )
                            # S = sum over heads (log tree), R = 1/S, P = E*R
                            E = Eb[:, kcl]
                            tmp = spool.tile([128, H // 2, QH], F16, tag="tmp")
                            nc.vector.tensor_tensor(
                                tmp, E[:, 0:8], E[:, 8:16], ADD
                            )
                            nc.vector.tensor_tensor(
                                tmp[:, 0:4], tmp[:, 0:4], tmp[:, 4:8], ADD
                            )
                            nc.vector.tensor_tensor(
                                tmp[:, 0:2], tmp[:, 0:2], tmp[:, 2:4], ADD
                            )
                            nc.vector.tensor_tensor(
                                tmp[:, 0:1], tmp[:, 0:1], tmp[:, 1:2], ADD
                            )
                            r = spool.tile([128, 1, QH], F16, tag="r")
                            with nc.allow_low_precision(
                                reason="softmax denominator reciprocal in fp16"
                            ):
                                nc.vector.reciprocal(r, tmp[:, 0:1])
                            nc.vector.tensor_tensor(
                                E[:, 0:8], E[:, 0:8],
                                r.to_broadcast([128, 8, QH]), MULT
                            )
                            nc.gpsimd.tensor_tensor(
                                E[:, 8:16], E[:, 8:16],
                                r.to_broadcast([128, 8, QH]), MULT
                            )
                        # attn^T: 4 waves x 2 d-chunks; one accumulation
                        # group per full PSUM bank (128 partitions), two
                        # zero-padded per-head matmuls per key chunk. 2-bank
                        # wave tiles with bufs=2 so the next wave's matmuls
                        # overlap this wave's VectorE spill-add.
                        for w in range(4):
                            ps = aps.tile([128, 2, 2 * QH], F32, tag="wv")
                            for kcl in range(KB):
                                kc = blk * KB + kcl
                                for jdl in range(2):
                                    for par in range(2):
                                        h = 4 * w + 2 * jdl + par
                                        lo = par * 64
                                        nc.tensor.matmul(
                                            ps[lo:lo + 64, jdl, 0:QH],
                                            lhsT=v_sb[:, kc,
                                                      h * 64:(h + 1) * 64],
                                            rhs=Eb[:, kcl, h, :],
                                            start=(kcl == 0),
                                            stop=(kcl == KB - 1),
                                            skip_group_check=True,
                                        )
                            if blk == 0:
                                nc.vector.tensor_copy(
                                    acc[:, 2 * w:2 * w + 2, :], ps[:, :, 0:QH]
                                )
                            elif blk == NBLK - 1:
                                # final spill writes the fp16 attn^T tile
                                # directly (saves a ScalarE conversion pass)
                                nc.vector.tensor_tensor(
                                    aT[:, 2 * w:2 * w + 2,
                                       qh * QH:(qh + 1) * QH],
                                    ps[:, :, 0:QH],
                                    acc[:, 2 * w:2 * w + 2, :],
                                    ADD,
                                )
                            else:
                                nc.vector.tensor_tensor(
                                    acc[:, 2 * w:2 * w + 2, :],
                                    ps[:, :, 0:QH],
                                    acc[:, 2 * w:2 * w + 2, :],
                                    ADD,
                                )


            # ---------------- Phase 3: output projection ----------------
            out_ch = chunked(out_d)  # [128, QS//128, D]
            with (
                tc.tile_pool(name="prj", bufs=2, space="PSUM") as prj,
                tc.tile_pool(name="outp", bufs=2) as opool,
            ):
                for qs in range(QS // 128):
                    for eh in range(2):
                        pm = prj.tile([128, 512], F32, tag="pm")
                        for jd in range(DC):
                            nc.tensor.matmul(
                                pm,
                                lhsT=aT[:, jd, qs * 128:(qs + 1) * 128],
                                rhs=wp_sb[:, jd, eh * 512:(eh + 1) * 512],
                                start=(jd == 0),
                                stop=(jd == DC - 1),
                            )
                        ot = opool.tile([128, 512], F32, tag="ot")
                        nc.vector.tensor_tensor(
                            ot, pm, bi_sb[:, eh * 512:(eh + 1) * 512], ADD
                        )
                        nc.sync.dma_start(
                            out_ch[:, qs, eh * 512:(eh + 1) * 512], ot
                        )

    nc.compile()
    return nc


def get_nc():
    global _CACHED_NC
    if _CACHED_NC is None:
        _CACHED_NC = _build_nc()
    return _CACHED_NC


def kernel(x, w_qkv, w_proj, b_proj, _trace=False, _tmpdir=None):
    x = np.asarray(x, dtype=np.float32)
    w_qkv = np.asarray(w_qkv, dtype=np.float32)
    w_proj = np.asarray(w_proj, dtype=np.float32)
    b_proj = np.asarray(b_proj, dtype=np.float32)

    # Host-side layout prep: transpose + fp16 casts + shard.
    xT = [np.ascontiguousarray(x[b].T).astype(np.float16) for b in range(B)]
    wq = np.ascontiguousarray(w_qkv[:, 0:D]).astype(np.float16)
    wk = np.ascontiguousarray(w_qkv[:, D:2 * D]).astype(np.float16)
    wv = np.ascontiguousarray(w_qkv[:, 2 * D:3 * D]).astype(np.float16)
    wp = w_proj.astype(np.float16)
    bias = np.ascontiguousarray(
        np.broadcast_to(b_proj, (128, D))
    ).astype(np.float32)

    in_maps = []
    for c in range(NCORES):
        b = c // (NCORES // B)
        qofs = (c % (NCORES // B)) * QS
        in_maps.append(
            {
                "xt": xT[b],
                "xtq": np.ascontiguousarray(xT[b][:, qofs:qofs + QS]),
                "wq": wq,
                "wk": wk,
                "wv": wv,
                "wp": wp,
                "bias": bias,
            }
        )

    nc = get_nc()
    res = bass_utils.run_bass_kernel_spmd(
        nc,
        in_maps,
        core_ids=list(range(NCORES)),
        trace=_trace,
        tmpdir=_tmpdir,
    )

    out = np.empty((B, T, D), dtype=np.float32)
    for c in range(NCORES):
        b = c // (NCORES // B)
        qofs = (c % (NCORES // B)) * QS
        out[b, qofs:qofs + QS] = res.results[c]["out"]
    if _trace:
        kernel._last_results = res
    return out
